# revision 14
# baseline (speedup 1.0000x reference)
"""CDWCE loss kernel for Trainium2 (8 NeuronCores, data-parallel over batch).

Math: loss = mean_b sum_j -log(1 - softmax(x)_bj + eps) * |j - t_b|^6
With u_bj = s_b - e_bj (s = row sum of exp), the per-element term is
v_bj = ln(s_b) - ln(u_bj)  (= -ln(1 - p_bj); the 1e-7 eps shifts the
reference value by <= ~1e-7 relative, far below tolerance).

|j-t|^6 is a degree-6 polynomial in t, so the dist-weighted sum over the
batch reduces to moment accumulations: with t' = t - 15.5, j' = j - 15.5,
  sum_b dist[t_b,j] * lnu_bj = sum_m w[m][j] * PS[m][j],
  PS[m][j] = sum_b t'_b^m * lnu_bj,   w[m][j] = C(6,m) (j')^(6-m) (-1)^m
and similarly R(t) = sum_j dist[t,j] for the ln(s) term. The moments are
computed on-chip by matmuls with a per-row powers matrix [1,t',..,t'^6]
as the stationary operand, accumulating in PSUM; the tiny combine happens
on the host in float64. (Empirically verified: bf16 moments + f32 PSUM
give ~1e-4 relative error on this distribution, vs 2e-2 tolerance.)

Engine split per tile: ACT exp+ln (one shared table set), DVE grouped
reduce + powers build + part of the subtract, GPSIMD the rest of the
broadcast-subtract u = s - e (runs concurrently with DVE), PE moment
matmuls.
"""

import numpy as np
from math import comb

B, C = 1048576, 32
N_CORES = 8
B_LOCAL = B // N_CORES          # 131072 rows per core
P = 128                         # SBUF partitions
G = 128                         # rows per partition per tile
NT = B_LOCAL // (P * G)         # 8 tiles per core
F = G * C                       # 4096 f32 per partition per tile
WG = G // 8                     # 16 subtract groups per tile (8 rows each)
QG = G // 16                    # 8 matmul groups per tile (16 rows each)
NCOL = 264                      # 8*32 ln(u) cols + 8 ln(s) cols per group
NPOW = 7                        # powers t'^0 .. t'^6
MROWS = 16 * NPOW               # 112 psum partition rows
TSHIFT = 15.5
ALPHA = 6
W_POOL = 10                     # subtract groups (of WG=16) done on GPSIMD

_PROG = None


def _patch_act_tables():
    """Force exp+ln onto the shared 'natural_log_exp_and_others' table set so
    interleaved exp/ln activations don't reload ACT tables every tile.
    Emptying the competing sets (instead of removing them) keeps
    act_func_set_id indices aligned with act_info.json."""
    import concourse.hw_specs as hw_specs
    from concourse import mybir

    if getattr(hw_specs.get_activation_tables, "_cdwce_patched", False):
        return
    AF = mybir.ActivationFunctionType
    orig = hw_specs.get_activation_tables

    def patched(arch):
        t = orig(arch)
        combined = "natural_log_exp_and_others"
        if combined in t and AF.Exp in t[combined] and AF.Ln in t[combined]:
            for k in list(t):
                if k != combined and (AF.Exp in t[k] or AF.Ln in t[k]):
                    t[k] = set()
        return t

    patched._cdwce_patched = True
    hw_specs.get_activation_tables = patched
    import concourse.bacc as bacc_mod

    if hasattr(bacc_mod, "get_activation_tables"):
        bacc_mod.get_activation_tables = patched


def _build_program():
    import concourse.bass as bass
    import concourse.bacc as bacc
    import concourse.tile as tile
    from concourse import mybir

    _patch_act_tables()
    AF = mybir.ActivationFunctionType
    Alu = mybir.AluOpType
    f32 = mybir.dt.float32
    bf16 = mybir.dt.bfloat16

    nc = bacc.Bacc("TRN2", target_bir_lowering=False, debug=False,
                   enable_asserts=True)
    x = nc.dram_tensor("x", [B_LOCAL, C], f32, kind="ExternalInput").ap()
    # t holds t' = targets - 15.5 (exact in bf16)
    t = nc.dram_tensor("t", [B_LOCAL], bf16, kind="ExternalInput").ap()
    out = nc.dram_tensor("out", [MROWS, 2, NCOL], f32,
                         kind="ExternalOutput").ap()

    # row index = n*(P*G) + p*G + g  ->  tile n, partition p, row-slot g
    xr = x.rearrange("(n p g) c -> n p (g c)", p=P, g=G)   # [NT, 128, F]
    tr = t.rearrange("(n p g) -> p n g", p=P, g=G)         # [128, NT, G]

    with tile.TileContext(nc) as tc:
        with (
            tc.tile_pool(name="consts", bufs=1) as consts,
            tc.tile_pool(name="xin", bufs=6) as xin,
            tc.tile_pool(name="work", bufs=3) as work,
            tc.tile_pool(name="smalls", bufs=3) as smalls,
            tc.tile_pool(name="psump", bufs=1, space="PSUM") as psump,
            tc.tile_pool(name="outp", bufs=1) as outp,
        ):
            t_sb = consts.tile([P, NT, G], bf16)
            nc.sync.dma_start(out=t_sb, in_=tr)

            # powers matrix for ALL tiles, built once:
            # tp_all[p, (n q), m, hh] = t'(tile n, row q*16+hh)^m
            tp_all = consts.tile([P, NT * QG, NPOW, 16], bf16)
            tv_all = t_sb.rearrange("p n (q hh) -> p (n q) hh", hh=16)
            nc.vector.memset(tp_all[:, :, 0, :], 1.0)
            nc.vector.tensor_copy(out=tp_all[:, :, 1, :], in_=tv_all)
            for m in range(2, NPOW):
                nc.vector.tensor_tensor(
                    out=tp_all[:, :, m, :], in0=tp_all[:, :, m - 1, :],
                    in1=tv_all, op=Alu.mult)

            psum_a = psump.tile([MROWS, NCOL], mybir.dt.float32)
            psum_b = psump.tile([MROWS, NCOL], mybir.dt.float32)

            state = {}

            def s_dma(i):
                xt = xin.tile([P, F], f32, name=f"xt{i}", tag="xt")
                nc.sync.dma_start(out=xt, in_=xr[i])
                state[i] = {"xt": xt}

            def s_exp(i):
                et = work.tile([P, F], f32, name=f"et{i}", tag="et")
                nc.scalar.activation(out=et, in_=state[i]["xt"], func=AF.Exp)
                state[i]["et"] = et

            def s_rsp(i):
                # reduce, broadcast-subtract (in place over et), powers matrix
                et = state[i]["et"]
                st = smalls.tile([P, G], f32, name=f"st{i}", tag="st")
                nc.vector.reduce_sum(
                    out=st,
                    in_=et.rearrange("p (g c) -> p g c", c=C),
                    axis=mybir.AxisListType.X,
                )
                s8 = st.rearrange("p (w h) -> p w h", h=8)        # [P,WG,8]
                s8b = s8.unsqueeze(3).to_broadcast([P, WG, 8, C])
                e8 = et.rearrange("p (w h c) -> p w h c", h=8, c=C)
                if W_POOL > 0:
                    nc.gpsimd.tensor_tensor(
                        out=e8[:, 0:W_POOL], in0=s8b[:, 0:W_POOL],
                        in1=e8[:, 0:W_POOL], op=Alu.subtract)
                if W_POOL < WG:
                    nc.vector.tensor_tensor(
                        out=e8[:, W_POOL:], in0=s8b[:, W_POOL:],
                        in1=e8[:, W_POOL:], op=Alu.subtract)

                state[i]["st"] = st

            def s_lnmm(i):
                et = state[i]["et"]
                st = state[i]["st"]
                e8 = et.rearrange("p (w h c) -> p w h c", h=8, c=C)
                s8 = st.rearrange("p (w h) -> p w h", h=8)
                ul = work.tile([P, WG, NCOL], bf16, name=f"ul{i}", tag="ul")
                nc.scalar.activation(
                    out=ul[:, :, 0:256].rearrange("p w (h c) -> p w h c", c=C),
                    in_=e8, func=AF.Ln)
                nc.scalar.activation(out=ul[:, :, 256:264], in_=s8, func=AF.Ln)
                for q in range(QG):
                    lhsT = tp_all[:, i * QG + q].rearrange("p m hh -> p (m hh)")
                    nc.tensor.matmul(
                        psum_a, lhsT, ul[:, 2 * q, :],
                        start=(i == 0 and q == 0),
                        stop=(i == NT - 1 and q == QG - 1),
                    )
                    nc.tensor.matmul(
                        psum_b, lhsT, ul[:, 2 * q + 1, :],
                        start=(i == 0 and q == 0),
                        stop=(i == NT - 1 and q == QG - 1),
                    )
                del state[i]

            # skewed software pipeline so no engine stream blocks another:
            # ACT sees exp(i) two tiles ahead of ln(i-2)
            for k in range(NT + 3):
                if k < NT:
                    s_dma(k)
                if 0 <= k - 1 < NT:
                    s_exp(k - 1)
                if 0 <= k - 2 < NT:
                    s_rsp(k - 2)
                if 0 <= k - 3 < NT:
                    s_lnmm(k - 3)

            out_sb = outp.tile([MROWS, 2, NCOL], f32)
            nc.vector.tensor_copy(out=out_sb[:, 0, :], in_=psum_a)
            nc.vector.tensor_copy(out=out_sb[:, 1, :], in_=psum_b)
            nc.sync.dma_start(out=out, in_=out_sb)

    nc.compile()
    return nc


def _get_program():
    global _PROG
    if _PROG is None:
        _PROG = _build_program()
    return _PROG


def _combine_tables():
    j = np.arange(C, dtype=np.float64)
    jp = j - TSHIFT
    w = np.zeros((NPOW, C))
    rw = np.zeros(NPOW)
    for m in range(NPOW):
        w[m] = comb(ALPHA, m) * jp ** (ALPHA - m) * (-1.0) ** m
        rw[m] = (comb(ALPHA, m) * jp ** (ALPHA - m)).sum() * (-1.0) ** m
    return w, rw


def _run(inputs, trace=False):
    import ml_dtypes
    from concourse.bass_utils import run_bass_kernel_spmd

    x_full = np.ascontiguousarray(np.asarray(inputs["outputs"], dtype=np.float32))
    t_full = np.asarray(inputs["targets"])
    assert x_full.shape == (B, C), x_full.shape
    tp_host = np.ascontiguousarray(
        (t_full.reshape(B).astype(np.float32) - TSHIFT).astype(ml_dtypes.bfloat16))

    xs = x_full.reshape(N_CORES, B_LOCAL, C)
    ts = tp_host.reshape(N_CORES, B_LOCAL)
    in_maps = [
        {"x": np.ascontiguousarray(xs[ci]), "t": np.ascontiguousarray(ts[ci])}
        for ci in range(N_CORES)
    ]

    nc = _get_program()
    res = run_bass_kernel_spmd(nc, in_maps, core_ids=list(range(N_CORES)),
                               trace=trace)

    pa = np.zeros((MROWS, NCOL), dtype=np.float64)
    pb = np.zeros((MROWS, NCOL), dtype=np.float64)
    for m in res.results:
        o = m["out"].astype(np.float64)
        pa += o[:, 0, :]
        pb += o[:, 1, :]

    # PS[m, j] = sum_b t'^m lnu[b, j]; PLS[m] = sum_b t'^m lns[b]
    ps = np.zeros((NPOW, C), dtype=np.float64)
    pls = np.zeros(NPOW, dtype=np.float64)
    par = pa.reshape(NPOW, 16, NCOL)
    pbr = pb.reshape(NPOW, 16, NCOL)
    for hh in range(8):
        ps += par[:, hh, 32 * hh:32 * (hh + 1)]
        pls += par[:, hh, 256 + hh]
    for hh in range(8, 16):
        ps += pbr[:, hh, 32 * (hh - 8):32 * (hh - 7)]
        pls += pbr[:, hh, 256 + hh - 8]

    w, rw = _combine_tables()
    loss = (np.dot(rw, pls) - np.sum(w * ps)) / B
    return np.float32(loss), res


def kernel(**inputs) -> np.ndarray:
    loss, _ = _run(inputs, trace=False)
    return np.asarray(loss, dtype=np.float32)


# revision 15
# speedup vs baseline: 1.0255x; 1.0255x over previous
"""CDWCE loss kernel for Trainium2 (8 NeuronCores, data-parallel over batch).

Math: loss = mean_b sum_j -log(1 - softmax(x)_bj + eps) * |j - t_b|^6
With u_bj = s_b - e_bj (s = row sum of exp), the per-element term is
v_bj = ln(s_b) - ln(u_bj)  (= -ln(1 - p_bj); the 1e-7 eps shifts the
reference value by <= ~1e-7 relative, far below tolerance).

|j-t|^6 is a degree-6 polynomial in t, so the dist-weighted sum over the
batch reduces to moment accumulations: with t' = t - 15.5, j' = j - 15.5,
  sum_b dist[t_b,j] * lnu_bj = sum_m w[m][j] * PS[m][j],
  PS[m][j] = sum_b t'_b^m * lnu_bj,   w[m][j] = C(6,m) (j')^(6-m) (-1)^m
and similarly R(t) = sum_j dist[t,j] for the ln(s) term. The moments are
computed on-chip by matmuls with a per-row powers matrix [1,t',..,t'^6]
as the stationary operand, accumulating in PSUM; the tiny combine happens
on the host in float64. (Empirically verified: bf16 moments + f32 PSUM
give ~1e-4 relative error on this distribution, vs 2e-2 tolerance.)

Engine split per tile: ACT exp+ln (one shared table set), DVE grouped
reduce + powers build + part of the subtract, GPSIMD the rest of the
broadcast-subtract u = s - e (runs concurrently with DVE), PE moment
matmuls.
"""

import numpy as np
from math import comb

B, C = 1048576, 32
N_CORES = 8
B_LOCAL = B // N_CORES          # 131072 rows per core
P = 128                         # SBUF partitions
G = 128                         # rows per partition per tile
NT = B_LOCAL // (P * G)         # 8 tiles per core
F = G * C                       # 4096 f32 per partition per tile
WG = G // 8                     # 16 subtract groups per tile (8 rows each)
QG = G // 16                    # 8 matmul groups per tile (16 rows each)
NCOL = 264                      # 8*32 ln(u) cols + 8 ln(s) cols per group
NPOW = 7                        # powers t'^0 .. t'^6
MROWS = 16 * NPOW               # 112 psum partition rows
TSHIFT = 15.5
ALPHA = 6
W_POOL = 10                     # subtract groups (of WG=16) done on GPSIMD

_PROG = None


def _patch_act_tables():
    """Force exp+ln onto the shared 'natural_log_exp_and_others' table set so
    interleaved exp/ln activations don't reload ACT tables every tile.
    Emptying the competing sets (instead of removing them) keeps
    act_func_set_id indices aligned with act_info.json."""
    import concourse.hw_specs as hw_specs
    from concourse import mybir

    if getattr(hw_specs.get_activation_tables, "_cdwce_patched", False):
        return
    AF = mybir.ActivationFunctionType
    orig = hw_specs.get_activation_tables

    def patched(arch):
        t = orig(arch)
        combined = "natural_log_exp_and_others"
        if combined in t and AF.Exp in t[combined] and AF.Ln in t[combined]:
            for k in list(t):
                if k != combined and (AF.Exp in t[k] or AF.Ln in t[k]):
                    t[k] = set()
        return t

    patched._cdwce_patched = True
    hw_specs.get_activation_tables = patched
    import concourse.bacc as bacc_mod

    if hasattr(bacc_mod, "get_activation_tables"):
        bacc_mod.get_activation_tables = patched


def _build_program():
    import concourse.bass as bass
    import concourse.bacc as bacc
    import concourse.tile as tile
    from concourse import mybir

    _patch_act_tables()
    AF = mybir.ActivationFunctionType
    Alu = mybir.AluOpType
    f32 = mybir.dt.float32
    bf16 = mybir.dt.bfloat16

    nc = bacc.Bacc("TRN2", target_bir_lowering=False, debug=False,
                   enable_asserts=True)
    x = nc.dram_tensor("x", [B_LOCAL, C], f32, kind="ExternalInput").ap()
    # t holds t' = targets - 15.5 (exact in bf16)
    t = nc.dram_tensor("t", [B_LOCAL], bf16, kind="ExternalInput").ap()
    out = nc.dram_tensor("out", [MROWS, 2, NCOL], f32,
                         kind="ExternalOutput").ap()

    # row index = n*(P*G) + p*G + g  ->  tile n, partition p, row-slot g
    xr = x.rearrange("(n p g) c -> n p (g c)", p=P, g=G)   # [NT, 128, F]
    tr = t.rearrange("(n p g) -> p n g", p=P, g=G)         # [128, NT, G]

    with tile.TileContext(nc) as tc:
        with (
            tc.tile_pool(name="consts", bufs=1) as consts,
            tc.tile_pool(name="xin", bufs=5) as xin,
            tc.tile_pool(name="work", bufs=4) as work,
            tc.tile_pool(name="smalls", bufs=3) as smalls,
            tc.tile_pool(name="psump", bufs=1, space="PSUM") as psump,
            tc.tile_pool(name="outp", bufs=1) as outp,
        ):
            t_sb = consts.tile([P, NT, G], bf16)
            nc.sync.dma_start(out=t_sb, in_=tr)

            # powers matrix for ALL tiles, built once:
            # tp_all[p, (n q), m, hh] = t'(tile n, row q*16+hh)^m
            tp_all = consts.tile([P, NT * QG, NPOW, 16], bf16)
            tv_all = t_sb.rearrange("p n (q hh) -> p (n q) hh", hh=16)
            nc.vector.memset(tp_all[:, :, 0, :], 1.0)
            nc.vector.tensor_copy(out=tp_all[:, :, 1, :], in_=tv_all)
            for m in range(2, NPOW):
                nc.vector.tensor_tensor(
                    out=tp_all[:, :, m, :], in0=tp_all[:, :, m - 1, :],
                    in1=tv_all, op=Alu.mult)

            psum_a = psump.tile([MROWS, NCOL], mybir.dt.float32)
            psum_b = psump.tile([MROWS, NCOL], mybir.dt.float32)

            state = {}

            def s_dma(i):
                xt = xin.tile([P, F], f32, name=f"xt{i}", tag="xt")
                nc.sync.dma_start(out=xt, in_=xr[i])
                state[i] = {"xt": xt}

            def s_exp(i):
                # et[:, w, 0:256] = exp(x); et[:, w, 256:264] = row sums later
                et = work.tile([P, WG, NCOL], f32, name=f"et{i}", tag="et")
                nc.scalar.activation(
                    out=et[:, :, 0:256].rearrange("p w (h c) -> p w h c", c=C),
                    in_=state[i]["xt"].rearrange("p (w h c) -> p w h c",
                                                 h=8, c=C),
                    func=AF.Exp)
                state[i]["et"] = et

            def s_rsp(i):
                # grouped row-sum into the tail columns, then in-place
                # broadcast-subtract u = s - e
                et = state[i]["et"]
                e8 = et[:, :, 0:256].rearrange("p w (h c) -> p w h c", c=C)
                s8 = et[:, :, 256:264]                            # [P,WG,8]
                nc.vector.reduce_sum(
                    out=s8, in_=e8, axis=mybir.AxisListType.X)
                s8b = s8.unsqueeze(3).to_broadcast([P, WG, 8, C])
                if W_POOL > 0:
                    nc.gpsimd.tensor_tensor(
                        out=e8[:, 0:W_POOL], in0=s8b[:, 0:W_POOL],
                        in1=e8[:, 0:W_POOL], op=Alu.subtract)
                if W_POOL < WG:
                    nc.vector.tensor_tensor(
                        out=e8[:, W_POOL:], in0=s8b[:, W_POOL:],
                        in1=e8[:, W_POOL:], op=Alu.subtract)

            def s_lnmm(i):
                et = state[i]["et"]
                ul = work.tile([P, WG, NCOL], bf16, name=f"ul{i}", tag="ul")
                nc.scalar.activation(out=ul, in_=et, func=AF.Ln)
                for q in range(QG):
                    lhsT = tp_all[:, i * QG + q].rearrange("p m hh -> p (m hh)")
                    nc.tensor.matmul(
                        psum_a, lhsT, ul[:, 2 * q, :],
                        start=(i == 0 and q == 0),
                        stop=(i == NT - 1 and q == QG - 1),
                    )
                    nc.tensor.matmul(
                        psum_b, lhsT, ul[:, 2 * q + 1, :],
                        start=(i == 0 and q == 0),
                        stop=(i == NT - 1 and q == QG - 1),
                    )
                del state[i]

            # skewed software pipeline so no engine stream blocks another:
            # ACT sees exp(i) three tiles ahead of ln(i-3)
            for k in range(NT + 4):
                if k < NT:
                    s_dma(k)
                if 0 <= k - 1 < NT:
                    s_exp(k - 1)
                if 0 <= k - 2 < NT:
                    s_rsp(k - 2)
                if 0 <= k - 4 < NT:
                    s_lnmm(k - 4)

            out_sb = outp.tile([MROWS, 2, NCOL], f32)
            nc.vector.tensor_copy(out=out_sb[:, 0, :], in_=psum_a)
            nc.vector.tensor_copy(out=out_sb[:, 1, :], in_=psum_b)
            nc.sync.dma_start(out=out, in_=out_sb)

    nc.compile()
    return nc


def _get_program():
    global _PROG
    if _PROG is None:
        _PROG = _build_program()
    return _PROG


def _combine_tables():
    j = np.arange(C, dtype=np.float64)
    jp = j - TSHIFT
    w = np.zeros((NPOW, C))
    rw = np.zeros(NPOW)
    for m in range(NPOW):
        w[m] = comb(ALPHA, m) * jp ** (ALPHA - m) * (-1.0) ** m
        rw[m] = (comb(ALPHA, m) * jp ** (ALPHA - m)).sum() * (-1.0) ** m
    return w, rw


def _run(inputs, trace=False):
    import ml_dtypes
    from concourse.bass_utils import run_bass_kernel_spmd

    x_full = np.ascontiguousarray(np.asarray(inputs["outputs"], dtype=np.float32))
    t_full = np.asarray(inputs["targets"])
    assert x_full.shape == (B, C), x_full.shape
    tp_host = np.ascontiguousarray(
        (t_full.reshape(B).astype(np.float32) - TSHIFT).astype(ml_dtypes.bfloat16))

    xs = x_full.reshape(N_CORES, B_LOCAL, C)
    ts = tp_host.reshape(N_CORES, B_LOCAL)
    in_maps = [
        {"x": np.ascontiguousarray(xs[ci]), "t": np.ascontiguousarray(ts[ci])}
        for ci in range(N_CORES)
    ]

    nc = _get_program()
    res = run_bass_kernel_spmd(nc, in_maps, core_ids=list(range(N_CORES)),
                               trace=trace)

    pa = np.zeros((MROWS, NCOL), dtype=np.float64)
    pb = np.zeros((MROWS, NCOL), dtype=np.float64)
    for m in res.results:
        o = m["out"].astype(np.float64)
        pa += o[:, 0, :]
        pb += o[:, 1, :]

    # PS[m, j] = sum_b t'^m lnu[b, j]; PLS[m] = sum_b t'^m lns[b]
    ps = np.zeros((NPOW, C), dtype=np.float64)
    pls = np.zeros(NPOW, dtype=np.float64)
    par = pa.reshape(NPOW, 16, NCOL)
    pbr = pb.reshape(NPOW, 16, NCOL)
    for hh in range(8):
        ps += par[:, hh, 32 * hh:32 * (hh + 1)]
        pls += par[:, hh, 256 + hh]
    for hh in range(8, 16):
        ps += pbr[:, hh, 32 * (hh - 8):32 * (hh - 7)]
        pls += pbr[:, hh, 256 + hh - 8]

    w, rw = _combine_tables()
    loss = (np.dot(rw, pls) - np.sum(w * ps)) / B
    return np.float32(loss), res


def kernel(**inputs) -> np.ndarray:
    loss, _ = _run(inputs, trace=False)
    return np.asarray(loss, dtype=np.float32)
